# revision 24
# baseline (speedup 1.0000x reference)
"""GQA attention (llama-style, RoPE, causal) on 8 Trainium2 NeuronCores.

Problem: B=2, S=2048, DIM=2048, 16 q-heads / 4 kv-heads, head_dim=128.

Sharding: batch x kv-group. Core c handles batch b=c//4 and kv-group
g=c%4 (q-heads 4g..4g+3, kv-head g). Each core computes its 4 heads'
attention and a partial output projection against wo[:, 512g:512(g+1)];
the host sums the 4 partials per batch. No cross-core communication.

Device-side layout is fully "transposed": activations live as [dim, seq]
so every matmul's contraction dim sits on the SBUF partition axis:
  qT/kT     = W^T-chunks @ xT           [head_dim, S]      (PSUM accum over din)
  rope      = qT*cos2 + (P@qT)*sin2     (partition-pair swap via a
                                         128x128 permutation matmul)
  scoresT   = kT-block^T-free @ qT      [kpos 128, q 512]
  expT      = Exp(scoresT * 1/sqrt(d))  (ScalarE, PSUM->SBUF)
  out^T     = sum_kb V-block @ expT     [head_dim, q]      (PSUM accum)
  sums      = ones^T @ esum             [1, q]   esum = 4-block DVE fold
  yT        = woT-chunks @ (out^T / sums)                  (PSUM accum)

All data tiles are fp16 (rel-err budget 2e-2; measured end-to-end
~1e-3): same 1 col/cycle PE stream rate as fp32r but FWL halves the
weight-load and DMA bytes halve. PE-column diet vs the fp32r original:
  - causal: k-blocks above the diagonal skipped; the 4 diagonal-overlap
    blocks per head-chunk run NARROWED (q columns >= 128j only), saving
    ~37k cols across scores/out; masking inside the triangle stays
    multiplicative post-exp against one shared [col>=p] triangle tile.
  - softmax sums: exp tiles folded 4:1 into the group-base tile on
    VectorE, then ONE ones-matmul per group (40 instead of 160 sum
    matmuls); the fold rides narrowed widths too.
Scheduling: emission is software-pipelined so the in-order PE queue
always has independent work queued ahead of every cross-engine
dependency — out-matmuls run one block behind their score/exp, each
group's sum-matmul four blocks behind its folds, each head's tail
(final out/sums/1-over-sums chain and the previous head's
normalization) flushes inside the NEXT head's 16-matmul projection
chain, and the y-projection runs h=0..2 partial accumulations two dm
ahead of the h=3+evict pass across three rotating psum banks. The next
s-chunk's x tiles prefetch during attention; a junk-matmul warmup burst
heats the PE clock (HAM) during the initial DMA ramp; DMAs sized
~128-512KB spread across the HWDGE queues; y evictions alternate
between VectorE and ScalarE into double-buffered fp16 slabs.
"""

import numpy as np
from contextlib import ExitStack

import bass_rust
import concourse.bass as bass
import concourse.mybir as mybir
import concourse.tile as tile
from concourse.bass_utils import run_bass_kernel_spmd

P = 128          # SBUF partitions / head_dim
S = 2048         # sequence length
D = 2048         # model dim
KC = 16          # contraction chunks of 128 over D
SC = 4           # s-chunks of 512
QW = 512         # moving-operand width
NH = 4           # q-heads per core
N_CORES = 8
SCALE = float(1.0 / np.sqrt(np.float32(128.0)))
F32 = mybir.dt.float32
F16 = mybir.dt.float16
F32R = mybir.dt.float32r
EXP = mybir.ActivationFunctionType.Exp
LN = mybir.ActivationFunctionType.Ln


class _TC(tile.TileContext):
    """TileContext whose tail drain splits its semaphore waits into
    separate wait instructions — the walrus build here rejects a Drain
    carrying more than a couple of inline sync waits."""

    def _drain_and_barrier(self, tick_clock, wait_clock):
        gc = tick_clock.global_clock
        ticks = [gc[i] for i in range(27)]
        for proc, sem in sorted(self.sems.allocated().items()):
            t = ticks[proc]
            if t > 0:
                mult = 16 if sem.name.startswith(("DMAHW", "DMASW")) else 1
                self.nc.sync.wait_ge(sem, t * mult)
        self.nc.sync.drain()
        self.nc.all_engine_barrier()
        popped = self.nc._tile_sem_poison_stack.pop()
        assert popped is self._sem_poison
        self.nc.clear_and_free_semaphores(list(self.sems.allocated().values()))
        self.nc.all_engine_barrier()


def _split_excess_waits(nc, max_waits=1):
    """This walrus build allows very few inline sync waits per TPB
    instruction. Move excess waits onto injected same-engine NOPs placed
    just before the instruction — semantically identical, since the
    engine queue executes in order."""
    for f in nc.m.functions:
        for blk in f.blocks:
            insts = blk.instructions
            new_list = []
            for inst in insts:
                si = inst.sync_info
                if si is not None and len(si.on_wait) > max_waits:
                    waits = list(si.on_wait)
                    excess, keep = waits[:-max_waits], waits[-max_waits:]
                    for j, w in enumerate(excess):
                        nop = bass_rust.InstNoOp(name=f"{inst.name}-wn{j}")
                        nop.engine = inst.engine
                        nop.sync_info = bass_rust.SyncInfo(
                            on_wait=[w], on_update=[])
                        new_list.append(nop)
                    inst.sync_info = bass_rust.SyncInfo(
                        on_wait=keep, on_update=list(si.on_update))
                new_list.append(inst)
            insts[:] = new_list


def _emit(nc, tc, ctx, t):
    pool = lambda name, bufs, space="SBUF": ctx.enter_context(
        tc.tile_pool(name=name, bufs=bufs, space=space)
    )

    # SBUF pools
    xp = pool("xp", 8)         # x chunk groups, 4 live + 4 prefetch
    constp = pool("constp", 1)  # weights, trig tables, masks, resident slabs
    csp = pool("csp", 2)       # cos/sin slices per s-chunk
    qsbp = pool("qsbp", 2)     # pre-rope proj copy
    t1p = pool("t1p", 2)
    t2p = pool("t2p", 2)
    qrp = pool("qrp", 4)       # rope'd q tiles, 4 live per q-chunk
    vsbp = pool("vsbp", 1)     # pre-transpose v copy
    ep = pool("ep", 10)        # exp tiles + out-of-place esums
    rp = pool("rp", 1)         # reciprocal [1, 512]
    rbp = pool("rbp", 2)       # broadcast recip [128, 512]
    otp = pool("otp", 5)       # normalized attention out, 4 live per q-chunk
    yp = pool("yp", 2)         # output copy slabs [128, 1024]

    # PSUM pools — 8 banks total
    pacc = pool("pacc", 2, "PSUM")    # proj accumulators      (2 banks)
    ptmp = pool("ptmp", 1, "PSUM")    # rope swap / v transpose / y  (1)
    pscore = pool("pscore", 2, "PSUM")  # scoresT              (2)
    pout = pool("pout", 2, "PSUM")    # attention out accum    (2)
    psmp = pool("psmp", 1, "PSUM")    # exp sums [1,512] / y   (1)

    # resident SBUF slabs. DMA emission order tracks first use: x(sc=0)
    # interleaved with wq quarters first, wo (needed ~100us in) last.
    wq_sb = constp.tile([P, KC * 4 * P], F16, tag="wq")   # chunk (k,h) at k*512+h*128
    wk_sb = constp.tile([P, KC * P], F16, tag="wk")       # chunk k at k*128
    wv_sb = constp.tile([P, KC * P], F16, tag="wv")
    wo_sb = constp.tile([P, NH * S], F16, tag="wo")       # chunk (h,dm) at h*2048+dm*128
    cm_sb = constp.tile([P, QW], F16, tag="cm")           # triangle mask col>=p
    perm_sb = constp.tile([P, P], F16, tag="perm")        # pair-swap permutation
    ident_sb = constp.tile([P, P], F16, tag="ident")
    ones_sb = constp.tile([P, 1], F16, tag="ones")
    onesrow_sb = constp.tile([1, P], F16, tag="onesrow")

    kT_sb = constp.tile([P, S], F16, tag="kT")    # rope'd K^T, filled per s-chunk
    vnat_sb = constp.tile([P, S], F16, tag="vn")  # V natural [kpos, d], 16 col-blocks

    xT_d, yT_d = t["xT"], t["yT"]

    def _load_xgroup(g, ssl, split=False):
        # one DMA covers 4 contraction chunks: [4,128,512] of x^T;
        # split per chunk at kernel start so chunk 0 lands fast via
        # parallel queues
        xg = xp.tile([P, 4 * QW], F16, tag="xg")
        if split:
            for j in range(4):
                nc.sync.dma_start(
                    xg[:, QW * j:QW * (j + 1)],
                    xT_d[P * (4 * g + j):P * (4 * g + j + 1), ssl])
        else:
            src = xT_d[4 * P * g:4 * P * (g + 1), ssl].rearrange(
                "(k p) s -> p k s", p=P)
            nc.sync.dma_start(xg[:].rearrange("p (k s) -> p k s", k=4), src)
        return xg

    def _const_dmas():
        nc.sync.dma_start(ones_sb[:], t["onescol"][:])
        nc.sync.dma_start(onesrow_sb[:], t["onesrow"][:])
        nc.sync.dma_start(cm_sb[:], t["cm"][:])
        for h in range(NH):
            nc.sync.dma_start(wo_sb[:, S * h:S * (h + 1)],
                              t["woT"][P * h:P * (h + 1), :])

    for sc in range(SC):
        ssl = slice(QW * sc, QW * (sc + 1))
        # ---- load x^T chunk groups (+ wq quarters on sc=0) ----
        if sc == 0:
            # tiny tensors first; junk matmuls on them heat the PE clock
            # (HAM) while the big DMA stream ramps
            nc.sync.dma_start(perm_sb[:], t["perm"][:])
            nc.sync.dma_start(ident_sb[:], t["ident"][:])
            wup = ptmp.tile([P, P], F32, tag="tmp")
            for _ in range(38):
                nc.tensor.matmul(wup[:], perm_sb[:], ident_sb[:],
                                 start=True, stop=True)
        if sc == 0:
            xgs = []
            for g in range(4):
                # wk quarters + x lead: the K projection runs first; wv
                # (needed only after the full K chain) follows the x stream
                wqt = slice(4 * P * g, 4 * P * (g + 1))
                nc.sync.dma_start(wk_sb[:, wqt], t["wk"][:, wqt])
                xgs.append(_load_xgroup(g, ssl, split=True))
            for g in range(4):
                wqt = slice(4 * P * g, 4 * P * (g + 1))
                nc.sync.dma_start(wv_sb[:, wqt], t["wv"][:, wqt])
            for h in range(4):
                for piece in range(2):  # 512KB pieces spread over queues
                    lo = h * 2048 + piece * 1024
                    nc.sync.dma_start(wq_sb[:, lo:lo + 1024],
                                      t["wq"][:, lo:lo + 1024])
        else:
            xgs = xgs_next
        xs = [xgs[k // 4][:, QW * (k % 4):QW * (k % 4 + 1)] for k in range(KC)]
        cos_t = csp.tile([P, QW], F16, tag="cos")
        nc.sync.dma_start(cos_t[:], t["cos2"][:, ssl])
        sin_t = csp.tile([P, QW], F16, tag="sin")
        nc.sync.dma_start(sin_t[:], t["sin2"][:, ssl])
        if sc == 0:
            _const_dmas()

        # ---- attention per head, interleaved with projections ----
        qc = sc
        nkb = 4 * qc + 4
        o_tiles = []
        pend_norm = []   # (po, r): norm awaiting the next tail flush
        tail = []        # per-head deferred closures (final out/sums/ln/r)

        def _flush_tail():
            while tail:
                tail.pop(0)()

        def _emit_norm(po, r, last=False):
            # broadcast 1/sums across partitions via a K=1 matmul; by the
            # time this runs on PE, r has long been ready (no PE stall).
            # The last head's norm runs inside the y-projection region, so
            # its psum comes from the (then idle) score pool — ptmp/psmp
            # are both mid-accumulation there and would deadlock the pools.
            rbp_ps = (pscore if last else ptmp).tile(
                [P, QW], F32, tag="score" if last else "tmp")
            nc.tensor.matmul(rbp_ps[:], onesrow_sb[:], r[:],
                             start=True, stop=True)
            rb = rbp.tile([P, QW], F16, tag="rb")
            nc.scalar.copy(rb[:], rbp_ps[:])
            ot = otp.tile([P, QW], F16, tag="ot")
            o_tiles.append(ot)
            nc.vector.tensor_mul(ot[:], po[:], rb[:])

        def _attn_head(qr, chain_mms=()):
            chain_mms = list(chain_mms)
            po = pout.tile([P, QW], F32, tag="out")
            psm = psmp.tile([1, QW], F32, tag="sum")
            esum = None
            out_pending = []   # out-MMs one block behind their score/exp
            sum_ready = []     # closed fold groups awaiting their sum-MM
            for kb in range(nkb):
                off = P * kb - QW * qc  # >=0 on diagonal-overlap blocks
                w = QW - max(off, 0)    # narrowed column count
                csl = slice(QW - w, QW)
                psc = pscore.tile([P, QW], F32, tag="score")
                nc.tensor.matmul(
                    psc[:, csl], kT_sb[:, P * kb:P * (kb + 1)],
                    qr[:, csl], start=True, stop=True,
                )
                if kb % 4 == 3 and len(sum_ready) > 1:
                    psm_, esum_, st_, sp_ = sum_ready.pop(0)
                    nc.tensor.matmul(psm_[:], ones_sb[:], esum_[:],
                                     start=st_, stop=sp_)
                # thread the next projection chain between score and out:
                # independent PE work that covers the exp latency
                for _ in range(-(-len(chain_mms) // (nkb - kb))):
                    chain_mms.pop(0)()
                if out_pending:
                    args, kw = out_pending.pop(0)
                    nc.tensor.matmul(*args, **kw)
                et = ep.tile([P, QW], F16, tag="exp")
                nc.scalar.activation(et[:, csl], psc[:, csl], EXP, scale=SCALE)
                if off >= 0:  # diagonal: zero kpos > q inside the block
                    nc.vector.tensor_mul(et[:, csl], et[:, csl], cm_sb[:, :w])
                out_pending.append((
                    (po[:, csl], vnat_sb[:, P * kb:P * (kb + 1)], et[:, csl]),
                    dict(start=(kb == 0), stop=(kb == nkb - 1)),
                ))
                if kb % 4 == 0:
                    esum = et  # group base is always full width (off<=0)
                else:
                    nc.vector.tensor_add(esum[:, csl], esum[:, csl], et[:, csl])
                if kb % 4 == 3:
                    sum_ready.append((psm, esum, kb == 3, kb == nkb - 1))
            # head tail (final out-MMs, last sum-MMs, 1/sums chain and the
            # PREVIOUS head's normalization) is deferred into the next
            # projection chain so the PE never waits on ScalarE/VectorE
            # latency at the head boundary.
            outs_left = list(out_pending)
            sums_left = list(sum_ready)

            def tail_fn(po=po, psm=psm, outs_left=outs_left, sums_left=sums_left):
                if pend_norm:
                    _emit_norm(*pend_norm.pop(0))
                for args, kw in outs_left:
                    nc.tensor.matmul(*args, **kw)
                for psm_, esum_, st_, sp_ in sums_left:
                    nc.tensor.matmul(psm_[:], ones_sb[:], esum_[:],
                                     start=st_, stop=sp_)
                # 1/sums = exp(-ln(sums)) on ScalarE (ACT-only)
                lnr = rp.tile([1, QW], F32, tag="r32")
                nc.scalar.activation(lnr[:], psm[:], LN)
                r = rp.tile([1, QW], F16, tag="r")
                nc.scalar.activation(r[:], lnr[:], EXP, scale=-1.0)
                pend_norm.append((po, r))

            tail.append(tail_fn)

        # ---- projections with software-pipelined rope/attention ----
        # Emission order gives every rope's perm-matmul and every head's
        # first score-matmul a full 16-MM projection chain of queued PE
        # work: chain(k), chain(v), rope(k), chain(q0), vtrans, rope(q0),
        # then per q-head: chain(q_{h+1}) / flush tail(h-1) / attn(h) /
        # rope(q_{h+1}).
        def _chain_mms(pi):
            ps = pacc.tile([P, QW], F32, tag="acc")
            thunks = []
            for k in range(KC):
                if pi < 4:
                    base = pi * 2048 + k * P
                    w_ap = wq_sb[:, base:base + P]
                elif pi == 4:
                    w_ap = wk_sb[:, k * P:(k + 1) * P]
                else:
                    w_ap = wv_sb[:, k * P:(k + 1) * P]
                thunks.append(lambda ps=ps, w_ap=w_ap, k=k: nc.tensor.matmul(
                    ps[:], w_ap, xs[k],
                    start=(k == 0), stop=(k == KC - 1),
                ))
            return ps, thunks

        def _chain(pi):
            ps, thunks = _chain_mms(pi)
            for th in thunks:
                th()
            return ps

        def _rope(pi, ps):
            # rope = psum*cos2 + (perm @ psum)*sin2
            qsb = qsbp.tile([P, QW], F16, tag="qsb")
            nc.scalar.copy(qsb[:], ps[:])
            sw = ptmp.tile([P, QW], F32, tag="tmp")
            nc.tensor.matmul(sw[:], perm_sb[:], qsb[:], start=True, stop=True)
            t1 = t1p.tile([P, QW], F16, tag="t1")
            nc.vector.tensor_mul(t1[:], qsb[:], cos_t[:])
            t2 = t2p.tile([P, QW], F16, tag="t2")
            nc.vector.tensor_mul(t2[:], sw[:], sin_t[:])
            if pi < 4:
                dst = qrp.tile([P, QW], F16, tag="qr")
                nc.vector.tensor_add(dst[:], t1[:], t2[:])
                return dst
            nc.vector.tensor_add(kT_sb[:, ssl], t1[:], t2[:])

        def _vtrans(ps):
            # V: psum -> sbuf, then PE-transpose 128-blocks into vnat
            vsb = vsbp.tile([P, QW], F16, tag="vsb")
            nc.scalar.copy(vsb[:], ps[:])
            for tb in range(4):
                pt = ptmp.tile([P, P], F16, tag="tmp")
                nc.tensor.transpose(pt[:], vsb[:, P * tb:P * (tb + 1)],
                                    ident_sb[:])
                blk = 4 * sc + tb
                nc.scalar.copy(vnat_sb[:, P * blk:P * (blk + 1)], pt[:])

        ps_k = _chain(4)
        ps_v = _chain(5)
        _rope(4, ps_k)
        ps = _chain(0)
        _vtrans(ps_v)
        qr_prev = _rope(0, ps)
        for pi in (1, 2, 3):
            ps, thunks = _chain_mms(pi)
            _flush_tail()
            _attn_head(qr_prev, thunks)
            qr_prev = _rope(pi, ps)
        _flush_tail()
        _attn_head(qr_prev)

        # ---- prefetch next chunk's x during the tail of attention ----
        if sc + 1 < SC:
            nssl = slice(QW * (sc + 1), QW * (sc + 2))
            xgs_next = [_load_xgroup(g, nssl, split=True) for g in range(4)]

        # ---- output projection for this q-chunk ----
        # y psums rotate through ptmp/psmp/pscore (pacc stays clear for the
        # next s-chunk's projections); eviction on DVE; stores batched 2
        # dm-tiles per DMA. The h=3 accumulation runs two dm behind h=0..2
        # so the last head's normalization — whose chain r -> bcast-matmul
        # -> DVE mul is ~2.5us deep — overlaps partial accumulations
        # instead of stalling the PE.
        _flush_tail()  # last head: final out/sums/ln/r + norm of head 2
        held_norm = pend_norm.pop(0)
        ypools = ((ptmp, "tmp"), (psmp, "sum"), (pscore, "score"),
                  (pacc, "acc"))
        pys = {}
        ysb = None

        def _wo_part(dm):
            pl, tg = ypools[dm % 4]
            py = pl.tile([P, QW], F32, tag=tg)
            pys[dm] = py
            for h in range(NH - 1):
                nc.tensor.matmul(
                    py[:], wo_sb[:, S * h + P * dm:S * h + P * (dm + 1)],
                    o_tiles[h][:], start=(h == 0), stop=False,
                )

        def _wo_fin(dm):
            nonlocal ysb
            py = pys.pop(dm)
            h = NH - 1
            nc.tensor.matmul(
                py[:], wo_sb[:, S * h + P * dm:S * h + P * (dm + 1)],
                o_tiles[h][:], start=False, stop=True,
            )
            if dm % 2 == 0:
                ysb = yp.tile([P, 2 * QW], F16, tag="y")
            if dm % 2 == 0:
                nc.vector.tensor_copy(ysb[:, :QW], py[:])
            else:
                nc.scalar.copy(ysb[:, QW:], py[:])
            if dm % 2 == 1:
                for j in range(2):
                    dmj = dm - 1 + j
                    nc.sync.dma_start(
                        yT_d[P * dmj:P * (dmj + 1), ssl],
                        ysb[:, QW * j:QW * (j + 1)])

        for dm in range(KC + 3):
            if dm < KC:
                _wo_part(dm)
            if dm == 1:
                _emit_norm(*held_norm, last=True)
            if dm >= 3:
                _wo_fin(dm - 3)


def build():
    nc = bass.Bass("TRN2", target_bir_lowering=False, debug=False,
                   num_devices=N_CORES)
    t = {
        "xT": nc.dram_tensor("xT", [D, S], F16, kind="ExternalInput"),
        "wq": nc.dram_tensor("wq", [P, KC * 4 * P], F16, kind="ExternalInput"),
        "wk": nc.dram_tensor("wk", [P, KC * P], F16, kind="ExternalInput"),
        "wv": nc.dram_tensor("wv", [P, KC * P], F16, kind="ExternalInput"),
        "woT": nc.dram_tensor("woT", [NH * P, S], F16, kind="ExternalInput"),
        "cos2": nc.dram_tensor("cos2", [P, S], F16, kind="ExternalInput"),
        "sin2": nc.dram_tensor("sin2", [P, S], F16, kind="ExternalInput"),
        "cm": nc.dram_tensor("cm", [P, QW], F16, kind="ExternalInput"),
        "perm": nc.dram_tensor("perm", [P, P], F16, kind="ExternalInput"),
        "ident": nc.dram_tensor("ident", [P, P], F16, kind="ExternalInput"),
        "onescol": nc.dram_tensor("onescol", [P, 1], F16, kind="ExternalInput"),
        "onesrow": nc.dram_tensor("onesrow", [1, P], F16, kind="ExternalInput"),
        "yT": nc.dram_tensor("yT", [D, S], F16, kind="ExternalOutput"),
    }
    aps = {k: v.ap() for k, v in t.items()}
    with _TC(nc, num_cores=N_CORES) as tc:
        with ExitStack() as ctx:
            _emit(nc, tc, ctx, aps)
    _split_excess_waits(nc)
    return nc


def host_inputs(x, wq, wk, wv, wo, freqs_cos, freqs_sin):
    """Shard + repack the full inputs into per-core in_maps."""
    f16 = np.float16
    cos2 = np.repeat(np.ascontiguousarray(freqs_cos.T), 2, axis=0).astype(f16)
    sin_t = np.ascontiguousarray(freqs_sin.T).astype(np.float32)
    sin2 = np.empty((P, S), np.float32)
    sin2[0::2] = -sin_t
    sin2[1::2] = sin_t
    sin2 = sin2.astype(f16)
    fidx = np.arange(QW)
    pidx = np.arange(P)
    cm = (fidx[None, :] >= pidx[:, None]).astype(f16)
    perm = np.zeros((P, P), np.float32)
    perm[pidx, pidx ^ 1] = 1.0
    perm = perm.astype(f16)
    ident = np.eye(P, dtype=f16)

    in_maps = []
    for c in range(N_CORES):
        b, g = divmod(c, 4)
        xT = np.ascontiguousarray(x[b].T).astype(f16)
        wq_s = wq[512 * g:512 * (g + 1)]                  # [512, 2048]
        wq_r = np.ascontiguousarray(
            wq_s.reshape(4, P, KC, P).transpose(3, 0, 2, 1).reshape(P, KC * 4 * P)
        ).astype(f16)
        wk_s = wk[P * g:P * (g + 1)]                      # [128, 2048]
        wk_r = np.ascontiguousarray(
            wk_s.reshape(P, KC, P).transpose(2, 1, 0).reshape(P, KC * P)
        ).astype(f16)
        wv_s = wv[P * g:P * (g + 1)]
        wv_r = np.ascontiguousarray(
            wv_s.reshape(P, KC, P).transpose(2, 1, 0).reshape(P, KC * P)
        ).astype(f16)
        woT = np.ascontiguousarray(wo[:, 512 * g:512 * (g + 1)].T).astype(f16)
        in_maps.append({
            "xT": xT, "wq": wq_r, "wk": wk_r, "wv": wv_r, "woT": woT,
            "cos2": cos2, "sin2": sin2, "cm": cm, "perm": perm, "ident": ident,
            "onescol": np.ones((P, 1), f16), "onesrow": np.ones((1, P), f16),
        })
    return in_maps


def combine_outputs(results):
    out = np.empty((2, S, D), np.float32)
    for b in range(2):
        acc = results[4 * b]["yT"].astype(np.float32)
        for g in range(1, 4):
            acc += results[4 * b + g]["yT"].astype(np.float32)
        out[b] = acc.T
    return out


_NC_CACHE = []


def kernel(x, wq, wk, wv, wo, freqs_cos, freqs_sin, mask):
    del mask  # causal structure handled on-device
    if not _NC_CACHE:
        _NC_CACHE.append(build())
    nc = _NC_CACHE[0]
    in_maps = host_inputs(x, wq, wk, wv, wo, freqs_cos, freqs_sin)
    res = run_bass_kernel_spmd(nc, in_maps, list(range(N_CORES)))
    return combine_outputs(res.results)
